# revision 1
# baseline (speedup 1.0000x reference)
"""Trainium2 Bass kernel for nn_ChemModel (DMPNN-style message-passing GNN).

Self-contained: call kernel(**inputs) with the full (unsharded) inputs from
setup_inputs(); returns the full [N_GRAPHS, 1] float32 output.

Strategy (8 NeuronCores, SPMD — one program, per-core data):
  * Nodes/slots sharded in contiguous dst ranges of N/8. The DMPNN
    recurrence only updates h rows [0, N) ("slots"); rows >= N are
    recomputed on the fly during the final aggregation.
  * Persistent transposed state hA_T [128h x SHP] lives in SBUF.
  * Per iteration: mA = relu(hA @ Wm^T) via one matmul per 128-slot block;
    shards AllGathered into a replicated DRAM table; per-edge messages
    fetched with two-phase dma_gather (int16 indices only reach 32K rows,
    so phase A groups edges by source-row range and lands them in a
    per-quarter message buffer; phase B re-gathers in dst-block order);
    segment-sum over dst is a one-hot matmul accumulated in PSUM per
    aligned 128-slot block, then added into hA_T.
  * Final pass: node embeddings with the same two-phase machinery
    (stream 0 gathers final hA rows for edge ids < N, stream 1 recomputes
    h0 = relu(xw[src] + ea @ Wie^T) for edge ids >= N), then
    relu(concat(x, node_emb) @ Wa^T), graph pooling through a 512-wide
    one-hot window per core, AllGather + baked-offset assembly of the
    pooled [G,128], and a small replicated FFN.
"""
import math
import numpy as np

import concourse.bass as bass
from concourse import bacc
import concourse.mybir as mybir
import concourse.tile as tile
from concourse.bass_utils import run_bass_kernel_spmd
from concourse import library_config

P = 128
NCORES = 8
GIDX_N = 2048              # indices per dma_gather instruction
GCH = GIDX_N // P          # chunks per gather instruction (16)
F32 = mybir.dt.float32
BF16 = mybir.dt.bfloat16
I16 = mybir.dt.int16
MSG_DT = BF16   # dtype of the hot-loop message tables
RELU = None                # set lazily (mybir enum)


def _relu():
    return mybir.ActivationFunctionType.Relu


def _copyf():
    return mybir.ActivationFunctionType.Copy


# ----------------------------------------------------------------------------
# host-side planning
# ----------------------------------------------------------------------------

def _wrap_idx16(flat):
    """[n] int array -> [128, n//16] int16 wrapped layout."""
    n = flat.shape[0]
    assert n % 16 == 0
    w = flat.reshape(n // 16, 16).T.astype(np.int16)
    return np.tile(w, (8, 1))


def _pad_to(arr, n, fill):
    out = np.full(n, fill, dtype=np.int64)
    out[:len(arr)] = np.asarray(arr, np.int64)
    return out


class _Plan:
    pass


def _host_prep(x, edge_index, edge_attr, batch, depth, G):
    N, E = x.shape[0], edge_index.shape[1]
    H = 128
    src = edge_index[0].astype(np.int64)
    dst = edge_index[1].astype(np.int64)
    batch = batch.astype(np.int64)

    assert N % NCORES == 0
    NSH = N // NCORES
    NB = math.ceil(NSH / P)
    SHP = NB * P
    TBL = NCORES * SHP
    RNG = 2 * SHP
    NRANGE = math.ceil(TBL / RNG)
    assert RNG <= 32512

    def rowof(v):
        return (v // NSH) * SHP + (v % NSH)

    row_src = rowof(src)

    qb = np.array_split(np.arange(NB), 4)

    plan = _Plan()
    plan.N, plan.E, plan.H, plan.G = N, E, H, G
    plan.NSH, plan.NB, plan.SHP, plan.TBL, plan.RNG, plan.NRANGE = \
        NSH, NB, SHP, TBL, RNG, NRANGE
    plan.depth = int(depth)
    plan.qb = qb
    plan.GW = min(512, G)

    # per-core edges sorted by local dst
    core_of = dst // NSH
    per_core = []
    for k in range(NCORES):
        eidx = np.where(core_of == k)[0]
        dloc = dst[eidx] - k * NSH
        order = np.argsort(dloc, kind="stable")
        per_core.append((eidx[order], dloc[order]))

    # global chunk capacity per block
    C = 1
    for k in range(NCORES):
        _, dloc = per_core[k]
        cnt = np.bincount(dloc // P, minlength=NB)
        C = max(C, int((cnt.max() + P - 1) // P))
    plan.C = C
    SLOTS = NB * C * P

    plan.nB_q = [math.ceil(len(qb[qi]) * C / GCH) for qi in range(4)]
    plan.padded_chunks_q = [plan.nB_q[qi] * GCH for qi in range(4)]
    plan.chunkblk_q = []
    for qi in range(4):
        cb = np.full(plan.padded_chunks_q[qi], -1, np.int64)
        for j, b in enumerate(qb[qi]):
            cb[j * C:(j + 1) * C] = b
        plan.chunkblk_q.append(cb)

    # per-core slot arrays (block-major chunk slots)
    cores = []
    for k in range(NCORES):
        eidx, dloc = per_core[k]
        info = {}
        slot_edge = np.full(SLOTS, -1, np.int64)
        cnt = np.bincount(dloc // P, minlength=NB)
        off = 0
        for b in range(NB):
            n_b = int(cnt[b])
            slot_edge[b * C * P:b * C * P + n_b] = eidx[off:off + n_b]
            off += n_b
        info["slot_edge"] = slot_edge
        sl_d = np.full(SLOTS, -1.0, np.float32)
        m = slot_edge >= 0
        sl_d[m] = (dst[slot_edge[m]] - k * NSH
                   - (np.arange(SLOTS)[m] // (C * P)) * P)
        info["dlocf"] = sl_d
        cores.append(info)

    def quarter_slots(qi):
        return np.concatenate([np.arange(b * C * P, (b + 1) * C * P)
                               for b in qb[qi]])

    def build_phase(key_rows_fn, streams):
        nin = [[[0] * NRANGE for _ in streams] for _ in range(4)]
        groups = [[None] * NCORES for _ in range(4)]
        for k in range(NCORES):
            info = cores[k]
            for qi in range(4):
                se = info["slot_edge"][quarter_slots(qi)]
                real = se[se >= 0]
                gs = []
                for si, (_, member) in enumerate(streams):
                    es = real[member(real)]
                    rows = key_rows_fn(si, es)
                    order = np.argsort(rows, kind="stable")
                    es, rows = es[order], rows[order]
                    rng = rows // RNG
                    per = [(es[rng == g], rows[rng == g]) for g in range(NRANGE)]
                    gs.append(per)
                    for g in range(NRANGE):
                        nin[qi][si][g] = max(
                            nin[qi][si][g],
                            math.ceil(max(len(per[g][0]), 1) / GIDX_N))
                groups[qi][k] = gs
        out = {"nin": nin}
        out["nA_q"] = [sum(nin[qi][si][g] for si in range(len(streams))
                           for g in range(NRANGE)) for qi in range(4)]
        for qi in range(4):
            assert out["nA_q"][qi] * GIDX_N <= 32512, \
                f"phase A msgbuf too large: {out['nA_q'][qi]}"
        percore = []
        for k in range(NCORES):
            info = cores[k]
            ga_cols, pb_cols = [], []
            ea_orders = [[] for _ in streams]
            for qi in range(4):
                pos = 0
                posmap = {}
                for si in range(len(streams)):
                    for g in range(NRANGE):
                        es, rows = groups[qi][k][si][g]
                        tot = nin[qi][si][g] * GIDX_N
                        idxs = np.zeros(tot, np.int64)
                        idxs[:len(rows)] = rows - g * RNG
                        for e_i, e in enumerate(es):
                            posmap[int(e)] = pos + e_i
                        ea_orders[si].append(_pad_to(es, tot, -1))
                        for ii in range(nin[qi][si][g]):
                            ga_cols.append(_wrap_idx16(
                                idxs[ii * GIDX_N:(ii + 1) * GIDX_N]))
                        pos += tot
                se = info["slot_edge"][quarter_slots(qi)]
                pi = np.zeros(len(se), np.int64)
                mreal = se >= 0
                if mreal.any():
                    pi[mreal] = np.array([posmap[int(e)] for e in se[mreal]],
                                         np.int64)
                pib = _pad_to(pi, plan.padded_chunks_q[qi] * P, 0)
                for ii in range(plan.nB_q[qi]):
                    pb_cols.append(_wrap_idx16(
                        pib[ii * GIDX_N:(ii + 1) * GIDX_N]))
            percore.append({
                "ga": np.concatenate(ga_cols, axis=1),
                "pb": np.concatenate(pb_cols, axis=1),
                "ea_orders": [np.concatenate(o) for o in ea_orders],
            })
        out["percore"] = percore
        return out

    plan.loop = build_phase(
        key_rows_fn=lambda si, es: row_src[es],
        streams=[("all", lambda es: np.ones(len(es), bool))])

    plan.fin = build_phase(
        key_rows_fn=lambda si, es: (rowof(es) if si == 0 else row_src[es]),
        streams=[("hfin", lambda es: es < N), ("h0", lambda es: es >= N)])

    # init phase (slot permutation gather of xw rows)
    initp = {"nin": [[0] * NRANGE]}
    init_groups = []
    for k in range(NCORES):
        sl = np.arange(k * NSH, (k + 1) * NSH)
        rows = row_src[sl]
        order = np.argsort(rows, kind="stable")
        sl_s, rows_s = sl[order], rows[order]
        rng = rows_s // RNG
        per = [(sl_s[rng == g], rows_s[rng == g]) for g in range(NRANGE)]
        init_groups.append(per)
        for g in range(NRANGE):
            initp["nin"][0][g] = max(
                initp["nin"][0][g],
                math.ceil((len(per[g][0]) + 1) / GIDX_N))
    initp["nA"] = sum(initp["nin"][0])
    assert initp["nA"] * GIDX_N <= 32512
    nBI = math.ceil(SHP // P / GCH)
    initp["nB"] = nBI
    initp["percore"] = []
    for k in range(NCORES):
        ga_cols = []
        posmap = np.zeros(NSH, np.int64)
        zpos, pos = None, 0
        for g in range(NRANGE):
            sls, rows = init_groups[k][g]
            tot = initp["nin"][0][g] * GIDX_N
            idxs = np.full(tot, NSH, np.int64)     # pads hit a zero row
            idxs[:len(rows)] = rows - g * RNG
            if zpos is None and len(rows) < tot:
                zpos = pos + len(rows)
            posmap[sls - k * NSH] = pos + np.arange(len(sls))
            for ii in range(initp["nin"][0][g]):
                ga_cols.append(_wrap_idx16(idxs[ii * GIDX_N:(ii + 1) * GIDX_N]))
            pos += tot
        assert zpos is not None
        pi = np.full(nBI * GIDX_N, zpos, np.int64)
        pi[:NSH] = posmap
        pb_cols = [_wrap_idx16(pi[ii * GIDX_N:(ii + 1) * GIDX_N])
                   for ii in range(nBI)]
        initp["percore"].append({
            "ga": np.concatenate(ga_cols, axis=1),
            "pb": np.concatenate(pb_cols, axis=1)})
    plan.init = initp

    # pooling bases
    plan.g_bases = []
    for k in range(NCORES):
        gb = int(batch[k * NSH])
        ge = int(batch[(k + 1) * NSH - 1])
        assert ge - gb < plan.GW, f"graph span {ge - gb} >= {plan.GW}"
        plan.g_bases.append(gb)

    # dense per-core inputs
    for k in range(NCORES):
        info = cores[k]
        xs = np.zeros((SHP, x.shape[1]), np.float32)
        xs[:NSH] = x[k * NSH:(k + 1) * NSH]
        info["xT"] = np.ascontiguousarray(xs.T)
        ei = np.zeros((SHP, edge_attr.shape[1]), np.float32)
        ei[:NSH] = edge_attr[k * NSH:(k + 1) * NSH]
        info["eaTi"] = np.ascontiguousarray(ei.T)
        o2 = plan.fin["percore"][k]["ea_orders"][1]
        ef = np.zeros((len(o2), edge_attr.shape[1]), np.float32)
        mreal = o2 >= 0
        ef[mreal] = edge_attr[o2[mreal]]
        info["eaTf"] = np.ascontiguousarray(ef.T)
        # padded per-quarter dlocf columns [128, sum(padded_chunks_q)]
        cols = []
        for qi in range(4):
            dq = info["dlocf"][quarter_slots(qi)].reshape(-1, P).T  # [128, nc]
            padc = plan.padded_chunks_q[qi] - dq.shape[1]
            cols.append(np.concatenate(
                [dq, np.full((P, padc), -1.0, np.float32)], axis=1))
        info["dlocf_in"] = np.ascontiguousarray(
            np.concatenate(cols, axis=1).astype(np.float32))
        bl = np.full((SHP,), -1.0, np.float32)
        bl[:NSH] = batch[k * NSH:(k + 1) * NSH] - plan.g_bases[k]
        info["batchloc"] = np.ascontiguousarray(bl.reshape(NB, P).T)
    plan.cores = cores

    plan.eaTf_cols = cores[0]["eaTf"].shape[1]
    for k in range(NCORES):
        assert cores[k]["eaTf"].shape[1] == plan.eaTf_cols
    plan.MSGROWS = max(max(plan.loop["nA_q"]), max(plan.fin["nA_q"]),
                       plan.init["nA"]) * GIDX_N
    return plan


# ----------------------------------------------------------------------------
# device kernel
# ----------------------------------------------------------------------------

def _build(plan, split=True):
    H, NB, C, SHP, TBL, RNG = (plan.H, plan.NB, plan.C, plan.SHP, plan.TBL,
                               plan.RNG)
    NRANGE, G, GW = plan.NRANGE, plan.G, plan.GW
    depth = plan.depth
    NPC = sum(plan.padded_chunks_q)

    nc = bacc.Bacc(num_devices=NCORES)

    def din(name, shape, dt=F32):
        return nc.declare_dram_parameter(name, list(shape), dt, isOutput=False)

    WmT = din("WmT", [H, H])
    WixT = din("WixT", [4, H])
    WieT = din("WieT", [3, H])
    WaxT = din("WaxT", [4, H])
    WahT = din("WahT", [H, H])
    W1T = din("W1T", [H, 4 * H])
    W2T = din("W2T", [4 * H, H])
    WlastT = din("WlastT", [H, 1])
    b1r = din("b1r", [H, 4])
    b2r = din("b2r", [H, 1])
    blast = din("blast", [1, 1])
    iota128 = din("iota128", [P, P])
    identb = din("identb", [P, P], BF16)
    iotaG = din("iotaG", [P, GW])
    ident = din("ident", [P, P])
    xT_in = din("xT", [4, SHP])
    eaTi_in = din("eaTi", [3, SHP])
    eaTf_in = din("eaTf", [3, plan.eaTf_cols])
    dlocf_in = din("dlocf", [P, NPC])
    batchloc_in = din("batchloc", [P, NB])

    nA_L = sum(plan.loop["nA_q"])
    nB_L = sum(plan.nB_q)
    nA_F = sum(plan.fin["nA_q"])
    nA_I = plan.init["nA"]
    nB_I = plan.init["nB"]
    gaL_in = din("gaL", [P, nA_L * (GIDX_N // 16)], I16)
    pbL_in = din("pbL", [P, nB_L * (GIDX_N // 16)], I16)
    gaF_in = din("gaF", [P, nA_F * (GIDX_N // 16)], I16)
    pbF_in = din("pbF", [P, nB_L * (GIDX_N // 16)], I16)
    gaI_in = din("gaI", [P, nA_I * (GIDX_N // 16)], I16)
    pbI_in = din("pbI", [P, nB_I * (GIDX_N // 16)], I16)

    out_ext = nc.declare_dram_parameter("out", [G, 1], F32, isOutput=True)

    RG = list(range(NCORES))

    with tile.TileContext(nc) as tc:
        nc.gpsimd.load_library(library_config.mlp)
        with (
            tc.tile_pool(name="cp", bufs=1) as cp,
            tc.tile_pool(name="sb", bufs=3) as sb,
            tc.tile_pool(name="ps", bufs=2, space="PSUM") as ps,
            tc.tile_pool(name="dr", bufs=1, space="DRAM") as dr,
        ):
            def cload(name, src):
                tl = cp.tile([src.shape[0], src.shape[1]], src.dtype, name=name)
                nc.sync.dma_start(out=tl[:], in_=src[:, :])
                return tl

            WmT_s = cload("WmT_s", WmT)
            WixT_s = cload("WixT_s", WixT)
            WieT_s = cload("WieT_s", WieT)
            WaxT_s = cload("WaxT_s", WaxT)
            WahT_s = cload("WahT_s", WahT)
            W1T_s = cload("W1T_s", W1T)
            W2T_f = []
            for f in range(4):
                tl = cp.tile([P, H], F32, name=f"W2T_{f}")
                nc.sync.dma_start(out=tl[:], in_=W2T[f * P:(f + 1) * P, :])
                W2T_f.append(tl)
            WlastT_s = cload("WlastT_s", WlastT)
            b1r_s = cload("b1r_s", b1r)
            b2r_s = cload("b2r_s", b2r)
            blast_s = cload("blast_s", blast)
            iota128_s = cload("iota128_s", iota128)
            iotaG_s = cload("iotaG_s", iotaG)
            ident_s = cload("ident_s", ident)
            identb_s = cload("identb_s", identb)
            dlocf_s = cload("dlocf_s", dlocf_in)
            batchloc_s = cload("batchloc_s", batchloc_in)
            gaI_s = cload("gaI_s", gaI_in)
            pbI_s = cload("pbI_s", pbI_in)

            def qidx(name, src_t, col0, ninst, tag):
                tl = sb.tile([P, ninst * (GIDX_N // 16)], I16, name=name,
                             tag=tag, bufs=2)
                nc.sync.dma_start(
                    out=tl[:],
                    in_=src_t[:, col0 * (GIDX_N // 16):
                              (col0 + ninst) * (GIDX_N // 16)])
                return tl

            hA_T = cp.tile([P, SHP], F32, name="hA_T")

            xw_local = dr.tile([SHP, H], MSG_DT, name="xw_local")
            xw_tbl = dr.tile([TBL, H], MSG_DT, name="xw_tbl",
                             addr_space="Shared")
            mA_local = dr.tile([SHP, H], MSG_DT, name="mA_local")
            mA_tbls = [dr.tile([TBL, H], MSG_DT, name=f"mA_tbl{it}",
                               addr_space="Shared") for it in range(depth)]
            hfin_local = dr.tile([SHP, H], MSG_DT, name="hfin_local")
            hfin_tbl = dr.tile([TBL, H], MSG_DT, name="hfin_tbl",
                               addr_space="Shared")
            gwin_local = dr.tile([P, GW], F32, name="gwin_local")
            gwin_all = dr.tile([NCORES * P, GW], F32, name="gwin_all",
                               addr_space="Shared")

            def allgather(local, table):
                nc.gpsimd.collective_compute(
                    "AllGather", mybir.AluOpType.bypass,
                    replica_groups=[RG], ins=[local[:]], outs=[table[:]])

            def gather(dst_ap, table_ap, idx_sb, inst_col):
                nc.gpsimd.dma_gather(
                    out_ap=dst_ap, in_ap=table_ap,
                    idxs_ap=idx_sb[:, inst_col * (GIDX_N // 16):
                                   (inst_col + 1) * (GIDX_N // 16)],
                    num_idxs=GIDX_N, num_idxs_reg=GIDX_N,
                    elem_size=H, single_packet=False)

            def iota3d(n):
                return bass.AP(iota128_s.tensor, iota128_s.offset,
                               [list(iota128_s.ap[0]), [0, n],
                                list(iota128_s.ap[1])])

            # ---------------- xw table ----------------
            for b in range(NB):
                xts = sb.tile([4, P], F32, name="xts")
                nc.sync.dma_start(out=xts[:], in_=xT_in[:, b * P:(b + 1) * P])
                pxw = ps.tile([P, H], F32, name="pxw", tag="ptmp", space="PSUM")
                nc.tensor.matmul(out=pxw[:], lhsT=xts[:], rhs=WixT_s[:],
                                 start=True, stop=True)
                txw = sb.tile([P, H], MSG_DT, name="txw", tag="sml")
                nc.vector.tensor_copy(out=txw[:], in_=pxw[:])
                nc.sync.dma_start(out=xw_local[b * P:(b + 1) * P, :], in_=txw[:])
            allgather(xw_local, xw_tbl)

            # ---------------- init hA_0 ----------------
            msgA = dr.tile([plan.MSGROWS, H], MSG_DT, name="msgA", tag="msg",
                           bufs=2)
            icol, pos = 0, 0
            for g in range(NRANGE):
                for _ in range(plan.init["nin"][0][g]):
                    gt = sb.tile([P, GCH, H], MSG_DT, name="gaIt", tag="gat")
                    gather(gt[:], xw_tbl[g * RNG:(g + 1) * RNG, :], gaI_s, icol)
                    icol += 1
                    nc.sync.dma_start(
                        out=msgA[pos:pos + GIDX_N, :].rearrange(
                            "(c p) h -> p c h", p=P),
                        in_=gt[:])
                    pos += GIDX_N
            for ii in range(nB_I):
                gt = sb.tile([P, GCH, H], MSG_DT, name="pbIt", tag="gat")
                gather(gt[:], msgA[:nA_I * GIDX_N, :], pbI_s, ii)
                eat = sb.tile([3, GIDX_N], F32, name="eati", tag="eat",
                              bufs=2)
                nc.sync.dma_start(
                    out=eat[:, :min(GIDX_N, SHP - ii * GIDX_N)],
                    in_=eaTi_in[:, ii * GIDX_N:min((ii + 1) * GIDX_N, SHP)])
                for c in range(GCH):
                    b = ii * GCH + c
                    if b >= NB:
                        break
                    ph = ps.tile([P, H], F32, name="ph0", tag="ptmp",
                                 space="PSUM")
                    nc.tensor.matmul(out=ph[:], lhsT=eat[:, c * P:(c + 1) * P],
                                     rhs=WieT_s[:], start=True, stop=False)
                    nc.tensor.matmul(out=ph[:], lhsT=identb_s[:],
                                     rhs=gt[:, c, :], start=False, stop=True)
                    t0 = sb.tile([P, H], F32, name="t0i", tag="sml")
                    nc.scalar.activation(out=t0[:], in_=ph[:], func=_relu())
                    pt = ps.tile([P, H], F32, name="pti", tag="ptmp",
                                 space="PSUM")
                    nc.tensor.transpose(out=pt[:], in_=t0[:],
                                        identity=ident_s[:])
                    nc.vector.tensor_copy(out=hA_T[:, b * P:(b + 1) * P],
                                          in_=pt[:])

            # ---------------- message-passing iterations ----------------
            qoff_chunks = [sum(plan.padded_chunks_q[:qi]) for qi in range(4)]
            for it in range(depth):
                for b in range(NB):
                    pm = ps.tile([P, H], F32, name="pm", tag="ptmp",
                                 space="PSUM")
                    nc.tensor.matmul(out=pm[:], lhsT=hA_T[:, b * P:(b + 1) * P],
                                     rhs=WmT_s[:], start=True, stop=True)
                    tm = sb.tile([P, H], MSG_DT, name="tm", tag="sml")
                    nc.scalar.activation(out=tm[:], in_=pm[:], func=_relu())
                    nc.sync.dma_start(out=mA_local[b * P:(b + 1) * P, :],
                                      in_=tm[:])
                mA_tbl = mA_tbls[it]
                allgather(mA_local, mA_tbl)

                acol, bcol = 0, 0
                for qi in range(4):
                    msg = dr.tile([plan.MSGROWS, H], MSG_DT, name="msgA",
                                  tag="msg", bufs=2)
                    gaq = qidx("gaLq", gaL_in, acol, plan.loop["nA_q"][qi],
                               "idxa")
                    pbq = qidx("pbLq", pbL_in, bcol, plan.nB_q[qi], "idxb")
                    qa = 0
                    pos = 0
                    for g in range(NRANGE):
                        for _ in range(plan.loop["nin"][qi][0][g]):
                            gt = sb.tile([P, GCH, H], MSG_DT, name="gaLt",
                                         tag="gat")
                            gather(gt[:], mA_tbl[g * RNG:(g + 1) * RNG, :],
                                   gaq, qa)
                            acol += 1
                            qa += 1
                            nc.sync.dma_start(
                                out=msg[pos:pos + GIDX_N, :].rearrange(
                                    "(c p) h -> p c h", p=P),
                                in_=gt[:])
                            pos += GIDX_N
                    chunkblk = plan.chunkblk_q[qi]
                    qb0 = int(plan.qb[qi][0])
                    open_pa = None
                    for ii in range(plan.nB_q[qi]):
                        gt = sb.tile([P, GCH, H], MSG_DT, name="pbLt",
                                     tag="gat")
                        gather(gt[:], msg[:plan.loop["nA_q"][qi] * GIDX_N, :],
                               pbq, ii)
                        oh = sb.tile([P, GCH, P], MSG_DT, name="ohL",
                                     tag="cmp")
                        gc0 = ii * GCH
                        dsl = dlocf_s[:, qoff_chunks[qi] + gc0:
                                      qoff_chunks[qi] + gc0 + GCH]
                        nc.vector.tensor_tensor(
                            out=oh[:], in0=dsl.to_broadcast([P, GCH, P]),
                            in1=iota3d(GCH), op=mybir.AluOpType.is_equal)
                        for c in range(GCH):
                            b = int(chunkblk[gc0 + c])
                            if b < 0:
                                continue
                            cin = (gc0 + c) - (b - qb0) * C
                            if cin == 0:
                                open_pa = ps.tile([P, H], F32, name="paL",
                                                  tag="pacc", space="PSUM")
                            nc.tensor.matmul(
                                out=open_pa[:], lhsT=gt[:, c, :],
                                rhs=oh[:, c, :],
                                start=(cin == 0), stop=(cin == C - 1))
                            if cin == C - 1:
                                nc.vector.tensor_add(
                                    out=hA_T[:, b * P:(b + 1) * P],
                                    in0=hA_T[:, b * P:(b + 1) * P],
                                    in1=open_pa[:])
                    bcol += plan.nB_q[qi]

            # ---------------- final aggregation ----------------
            for b in range(NB):
                pt = ps.tile([P, H], F32, name="ptf", tag="ptmp", space="PSUM")
                nc.tensor.transpose(out=pt[:], in_=hA_T[:, b * P:(b + 1) * P],
                                    identity=ident_s[:])
                tf = sb.tile([P, H], MSG_DT, name="tf", tag="sml")
                nc.vector.tensor_copy(out=tf[:], in_=pt[:])
                nc.sync.dma_start(out=hfin_local[b * P:(b + 1) * P, :],
                                  in_=tf[:])
            allgather(hfin_local, hfin_tbl)

            gps = ps.tile([P, GW], F32, name="gps", tag="gps", space="PSUM",
                          bufs=1)
            acol, bcol, ea_col = 0, 0, 0
            pool_started = False
            for qi in range(4):
                msg = dr.tile([plan.MSGROWS, H], MSG_DT, name="msgA",
                              tag="msg", bufs=2)
                gaq = qidx("gaFq", gaF_in, acol, plan.fin["nA_q"][qi], "idxa")
                pbq = qidx("pbFq", pbF_in, bcol, plan.nB_q[qi], "idxb")
                qa = 0
                pos = 0
                for g in range(NRANGE):
                    for _ in range(plan.fin["nin"][qi][0][g]):
                        gt = sb.tile([P, GCH, H], MSG_DT, name="gaF0t",
                                     tag="gat")
                        gather(gt[:], hfin_tbl[g * RNG:(g + 1) * RNG, :],
                               gaq, qa)
                        acol += 1
                        qa += 1
                        nc.sync.dma_start(
                            out=msg[pos:pos + GIDX_N, :].rearrange(
                                "(c p) h -> p c h", p=P),
                            in_=gt[:])
                        pos += GIDX_N
                for g in range(NRANGE):
                    for _ in range(plan.fin["nin"][qi][1][g]):
                        gt = sb.tile([P, GCH, H], MSG_DT, name="gaF1t",
                                     tag="gat")
                        gather(gt[:], xw_tbl[g * RNG:(g + 1) * RNG, :],
                               gaq, qa)
                        acol += 1
                        qa += 1
                        eat = sb.tile([3, GIDX_N], F32, name="eatf",
                                      tag="eat", bufs=2)
                        nc.sync.dma_start(
                            out=eat[:],
                            in_=eaTf_in[:, ea_col * GIDX_N:
                                        (ea_col + 1) * GIDX_N])
                        ea_col += 1
                        ot = sb.tile([P, GCH, H], MSG_DT, name="h0t",
                                     tag="cmp")
                        for c in range(GCH):
                            ph = ps.tile([P, H], F32, name="ph0f", tag="ptmp",
                                         space="PSUM")
                            nc.tensor.matmul(
                                out=ph[:], lhsT=eat[:, c * P:(c + 1) * P],
                                rhs=WieT_s[:], start=True, stop=False)
                            nc.tensor.matmul(out=ph[:], lhsT=identb_s[:],
                                             rhs=gt[:, c, :], start=False,
                                             stop=True)
                            nc.scalar.activation(out=ot[:, c, :], in_=ph[:],
                                                 func=_relu())
                        nc.sync.dma_start(
                            out=msg[pos:pos + GIDX_N, :].rearrange(
                                "(c p) h -> p c h", p=P),
                            in_=ot[:])
                        pos += GIDX_N
                chunkblk = plan.chunkblk_q[qi]
                qb0 = int(plan.qb[qi][0])
                open_pa = None
                for ii in range(plan.nB_q[qi]):
                    gt = sb.tile([P, GCH, H], MSG_DT, name="pbFt",
                                 tag="gat")
                    gather(gt[:], msg[:plan.fin["nA_q"][qi] * GIDX_N, :],
                           pbq, ii)
                    oh = sb.tile([P, GCH, P], MSG_DT, name="ohF", tag="cmp")
                    gc0 = ii * GCH
                    dsl = dlocf_s[:, qoff_chunks[qi] + gc0:
                                  qoff_chunks[qi] + gc0 + GCH]
                    nc.vector.tensor_tensor(
                        out=oh[:], in0=dsl.to_broadcast([P, GCH, P]),
                        in1=iota3d(GCH), op=mybir.AluOpType.is_equal)
                    for c in range(GCH):
                        b = int(chunkblk[gc0 + c])
                        if b < 0:
                            continue
                        cin = (gc0 + c) - (b - qb0) * C
                        if cin == 0:
                            open_pa = ps.tile([P, H], F32, name="paF",
                                              tag="pacc", space="PSUM")
                        nc.tensor.matmul(
                            out=open_pa[:], lhsT=oh[:, c, :], rhs=gt[:, c, :],
                            start=(cin == 0), stop=(cin == C - 1))
                        if cin == C - 1:
                            tnm = sb.tile([P, H], F32, name="tnm", tag="sml")
                            nc.vector.tensor_copy(out=tnm[:], in_=open_pa[:])
                            ptr = ps.tile([P, H], F32, name="ptrF", tag="ptmp",
                                          space="PSUM")
                            nc.tensor.transpose(out=ptr[:], in_=tnm[:],
                                                identity=ident_s[:])
                            tT = sb.tile([P, H], F32, name="tTf", tag="sml")
                            nc.vector.tensor_copy(out=tT[:], in_=ptr[:])
                            xts = sb.tile([4, P], F32, name="xts")
                            nc.sync.dma_start(
                                out=xts[:], in_=xT_in[:, b * P:(b + 1) * P])
                            p2 = ps.tile([P, H], F32, name="p2f", tag="ptmp",
                                         space="PSUM")
                            nc.tensor.matmul(out=p2[:], lhsT=tT[:],
                                             rhs=WahT_s[:], start=True,
                                             stop=False)
                            nc.tensor.matmul(out=p2[:], lhsT=xts[:],
                                             rhs=WaxT_s[:], start=False,
                                             stop=True)
                            ne2 = sb.tile([P, H], F32, name="ne2", tag="sml")
                            nc.scalar.activation(out=ne2[:], in_=p2[:],
                                                 func=_relu())
                            ohg = sb.tile([P, GW], F32, name="ohg", tag="ohg")
                            nc.vector.tensor_tensor(
                                out=ohg[:],
                                in0=batchloc_s[:, b:b + 1].to_broadcast(
                                    [P, GW]),
                                in1=iotaG_s[:],
                                op=mybir.AluOpType.is_equal)
                            nc.tensor.matmul(
                                out=gps[:], lhsT=ne2[:], rhs=ohg[:],
                                start=(not pool_started), stop=(b == NB - 1))
                            pool_started = True
                bcol += plan.nB_q[qi]

            tgw = sb.tile([P, GW], F32, name="tgw", tag="ohg")
            nc.vector.tensor_copy(out=tgw[:], in_=gps[:])
            nc.sync.dma_start(out=gwin_local[:, :], in_=tgw[:])
            allgather(gwin_local, gwin_all)
            gfull = cp.tile([P, G], F32, name="gfull")
            nc.vector.memset(gfull[:], 0.0)
            for j in range(NCORES):
                wj = min(GW, G - plan.g_bases[j])
                tw = sb.tile([P, GW], F32, name="twj", tag="ohg")
                nc.sync.dma_start(out=tw[:], in_=gwin_all[j * P:(j + 1) * P, :])
                nc.vector.tensor_add(
                    out=gfull[:, plan.g_bases[j]:plan.g_bases[j] + wj],
                    in0=gfull[:, plan.g_bases[j]:plan.g_bases[j] + wj],
                    in1=tw[:, :wj])

            # ---------------- FFN (replicated on all cores) ----------------
            NGC = math.ceil(G / 512)
            z2sb = cp.tile([P, G], F32, name="z2sb")
            nc.vector.memset(z2sb[:], 0.0)
            for f in range(4):
                z1f = sb.tile([P, G], F32, name="z1f", tag="z1f", bufs=2)
                for gc in range(NGC):
                    g0, g1 = gc * 512, min((gc + 1) * 512, G)
                    pz = ps.tile([P, 512], F32, name="pz", tag="ptmp",
                                 space="PSUM")
                    nc.tensor.matmul(out=pz[:, :g1 - g0],
                                     lhsT=W1T_s[:, f * P:(f + 1) * P],
                                     rhs=gfull[:, g0:g1], start=True,
                                     stop=True)
                    nc.scalar.activation(out=z1f[:, g0:g1], in_=pz[:, :g1 - g0],
                                         func=_relu(), bias=b1r_s[:, f:f + 1])
                for gc in range(NGC):
                    g0, g1 = gc * 512, min((gc + 1) * 512, G)
                    pz2 = ps.tile([P, 512], F32, name="pz2", tag="ptmp",
                                  space="PSUM")
                    nc.tensor.matmul(out=pz2[:, :g1 - g0], lhsT=W2T_f[f][:],
                                     rhs=z1f[:, g0:g1], start=True, stop=True)
                    nc.vector.tensor_add(out=z2sb[:, g0:g1],
                                         in0=z2sb[:, g0:g1],
                                         in1=pz2[:, :g1 - g0])
            nc.vector.tensor_add(out=z2sb[:], in0=z2sb[:],
                                 in1=b2r_s[:, 0:1].to_broadcast([P, G]))
            orow = sb.tile([1, G], F32, name="orow", tag="z1f", bufs=2)
            for gc in range(NGC):
                g0, g1 = gc * 512, min((gc + 1) * 512, G)
                po = ps.tile([1, 512], F32, name="po", tag="po", space="PSUM")
                nc.tensor.matmul(out=po[:, :g1 - g0], lhsT=WlastT_s[:],
                                 rhs=z2sb[:, g0:g1], start=True, stop=True)
                nc.vector.tensor_add(
                    out=orow[:, g0:g1], in0=po[:, :g1 - g0],
                    in1=blast_s[0:1, 0:1].to_broadcast([1, g1 - g0]))
            nc.sync.dma_start(out=out_ext[:, :], in_=orow[:])

    nc.compile()
    if split:
        _split_excess_waits(nc)
    return nc


def _split_excess_waits(nc, max_waits=1):
    k = 0
    for f in nc.m.functions:
        for bb in f.blocks:
            new = []
            for ins in bb.instructions:
                si = ins.sync_info
                if si is not None and len(si.on_wait) > max_waits:
                    waits = list(si.on_wait)
                    for w in waits[:-max_waits]:
                        nop = mybir.InstNoOp(name=f"I-waitsplit-{k}",
                                             engine=ins.engine)
                        k += 1
                        nop.sync_info = mybir.SyncInfo(on_wait=[w],
                                                       on_update=[])
                        new.append(nop)
                    si.on_wait = waits[-max_waits:]
                new.append(ins)
            bb.instructions = new
    return k


# ----------------------------------------------------------------------------
# inputs
# ----------------------------------------------------------------------------

def _in_maps(plan, weights):
    H = plan.H
    GW = plan.GW
    com = {
        "WmT": np.ascontiguousarray(weights["W_m"].T),
        "WixT": np.ascontiguousarray(weights["W_i"][:, :4].T),
        "WieT": np.ascontiguousarray(weights["W_i"][:, 4:].T),
        "WaxT": np.ascontiguousarray(weights["W_a"][:, :4].T),
        "WahT": np.ascontiguousarray(weights["W_a"][:, 4:].T),
        "W1T": np.ascontiguousarray(weights["W1"].T),
        "W2T": np.ascontiguousarray(weights["W2"].T),
        "WlastT": np.ascontiguousarray(weights["W_last"].T),
        "b1r": np.ascontiguousarray(weights["b1"].reshape(4, H).T),
        "b2r": weights["b2"].reshape(H, 1).copy(),
        "blast": weights["b_last"].reshape(1, 1).copy(),
        "iota128": np.tile(np.arange(P, dtype=np.float32), (P, 1)),
        "iotaG": np.tile(np.arange(GW, dtype=np.float32), (P, 1)),
        "ident": np.eye(P, dtype=np.float32),
    }
    import ml_dtypes
    com["identb"] = np.eye(P).astype(ml_dtypes.bfloat16)
    maps = []
    for k in range(NCORES):
        info = plan.cores[k]
        m = dict(com)
        m["xT"] = info["xT"]
        m["eaTi"] = info["eaTi"]
        m["eaTf"] = info["eaTf"]
        m["dlocf"] = info["dlocf_in"]
        m["batchloc"] = info["batchloc"]
        m["gaL"] = plan.loop["percore"][k]["ga"]
        m["pbL"] = plan.loop["percore"][k]["pb"]
        m["gaF"] = plan.fin["percore"][k]["ga"]
        m["pbF"] = plan.fin["percore"][k]["pb"]
        m["gaI"] = plan.init["percore"][k]["ga"]
        m["pbI"] = plan.init["percore"][k]["pb"]
        maps.append(m)
    return maps


def _prep_all(x, edge_index, edge_attr, batch, depth, weights, G):
    plan = _host_prep(np.asarray(x, np.float32), np.asarray(edge_index),
                      np.asarray(edge_attr, np.float32), np.asarray(batch),
                      int(depth), G)
    maps = _in_maps(plan, weights)
    return plan, maps


def kernel(x, edge_index, edge_attr, batch, depth,
           W_i, W_m, W_a, W1, b1, W2, b2, W_last, b_last):
    weights = {
        "W_i": np.asarray(W_i, np.float32), "W_m": np.asarray(W_m, np.float32),
        "W_a": np.asarray(W_a, np.float32), "W1": np.asarray(W1, np.float32),
        "b1": np.asarray(b1, np.float32), "W2": np.asarray(W2, np.float32),
        "b2": np.asarray(b2, np.float32),
        "W_last": np.asarray(W_last, np.float32),
        "b_last": np.asarray(b_last, np.float32),
    }
    G = 2048
    plan, maps = _prep_all(x, edge_index, edge_attr, batch, depth, weights, G)
    nc = _build(plan, split=True)
    res = run_bass_kernel_spmd(nc, maps, list(range(NCORES)))
    return np.asarray(res.results[0]["out"]).reshape(G, 1).astype(np.float32)



# revision 9
# speedup vs baseline: 5.2787x; 5.2787x over previous
"""Trainium2 Bass kernel for nn_ChemModel (DMPNN-style message-passing GNN).

Self-contained: call kernel(**inputs) with the full (unsharded) inputs from
setup_inputs(); returns the full [N_GRAPHS, 1] float32 output.

Strategy (8 NeuronCores, SPMD — one program, per-core data):
  * Nodes/slots sharded by dst owner in contiguous ranges of N/8. Only h rows
    [0, N) evolve; rows >= N keep h0 and are recomputed during the final
    aggregation from host-packed [x[src]|edge_attr] columns (no gather).
  * Persistent transposed state hA_T [128h x SHP] in SBUF.
  * Message tables live in DRAM with a partition-major row order
    (row = core*SHP + (slot%128)*NB + slot//128) so the local shard is
    written with ONE contiguous DMA, then AllGathered (bf16).
  * Per-edge messages are fetched with a SINGLE dma_gather pass directly in
    dst-sorted order: edges are grouped by table-row % 4 and gathered with
    elem_step=4*H (int16 indices then cover the whole 100K-row table).
  * Scatter-add is one-hot matmul into sliding 256-slot PSUM windows with a
    host-computed, core-uniform window schedule (max over cores); padding
    edges carry dloc=-1 and self-neutralize in the one-hot.
  * Final phase: h-final rows for edge ids < N come from one more
    table+gather pass; edge ids >= N recompute h0 via matmul from host-packed
    inputs, overlapped with the collective + gathers.
"""
import math
import numpy as np
import ml_dtypes

import concourse.bass as bass
from concourse import bacc
import concourse.mybir as mybir
import concourse.tile as tile
from concourse.bass_utils import run_bass_kernel_spmd
from concourse import library_config

P = 128
H = 128
NCORES = 8
GIDX_N = 2048              # indices per dma_gather instruction
GCH = GIDX_N // P          # chunks per gather instruction (16)
ICOL = GIDX_N // 16        # idx columns per instruction in wrapped layout
WSL = 256                  # scatter window width in slots (2 blocks)
NRES = 4                   # residue groups (table row % 4)
JMAX = 8                   # max windows spanned by one chunk
F32 = mybir.dt.float32
BF16 = mybir.dt.bfloat16
I16 = mybir.dt.int16
BF16NP = ml_dtypes.bfloat16


def _relu():
    return mybir.ActivationFunctionType.Relu


def _wrap_idx16(flat):
    """[n] int array -> [128, n//16] int16 wrapped layout."""
    n = flat.shape[0]
    assert n % 16 == 0
    w = flat.reshape(n // 16, 16).T.astype(np.int16)
    return np.tile(w, (8, 1))


class _Plan:
    pass


def _build_stream_groups(rows_key, dloc, extra=None):
    """Split one core's edge stream into NRES residue groups sorted by dloc.

    Returns per group dict with idx ((row - r) // 4), dloc, and optionally a
    sorted copy of `extra` (per-edge payload columns [n, width])."""
    out = []
    for r in range(NRES):
        m = (rows_key % NRES) == r
        dl = dloc[m]
        rw = rows_key[m]
        o = np.argsort(dl, kind="stable")
        g = {"idx": (rw[o] - r) // NRES, "dloc": dl[o]}
        if extra is not None:
            g["extra"] = extra[m][o]
        out.append(g)
    return out


def _chunk_minmax(dloc_pad):
    """dloc_pad: [nch, P] with -1 pads -> per-chunk (min, max) over real
    entries; (inf, -inf) when empty."""
    real = dloc_pad >= 0
    mn = np.where(real, dloc_pad, np.inf).min(axis=1)
    mx = np.where(real, dloc_pad, -np.inf).max(axis=1)
    return mn, mx


def _host_prep(x, edge_index, edge_attr, batch, depth, G):
    N, E = x.shape[0], edge_index.shape[1]
    src = edge_index[0].astype(np.int64)
    dst = edge_index[1].astype(np.int64)
    batch = batch.astype(np.int64)
    x = np.asarray(x, np.float32)
    ea = np.asarray(edge_attr, np.float32)

    assert N % NCORES == 0
    NSH = N // NCORES
    NB = math.ceil(NSH / P)
    SHP = NB * P
    TBL = NCORES * SHP
    assert TBL % NRES == 0 and TBL // NRES <= 32512
    assert SHP % WSL == 0
    NW = SHP // WSL

    plan = _Plan()
    plan.N, plan.E, plan.G = N, E, G
    plan.NSH, plan.NB, plan.SHP, plan.TBL = NSH, NB, SHP, TBL
    plan.NW = NW
    plan.depth = int(depth)
    plan.GW = min(512, G)

    def row_of(n):
        s = n % NSH
        return (n // NSH) * SHP + (s % P) * NB + s // P

    row_src = row_of(src)

    # per-core edge partitions by dst owner
    core_of = dst // NSH
    per_core_loop = []   # all E edges (loop + used again conceptually)
    per_core_f0 = []     # edge ids < N  (gather h_final rows)
    per_core_f1 = []     # edge ids >= N (recompute h0)
    eids = np.arange(E)
    for k in range(NCORES):
        m = core_of == k
        ek = eids[m]
        dl = dst[m] - k * NSH
        per_core_loop.append(
            _build_stream_groups(row_src[ek], dl))
        m0 = ek < N
        per_core_f0.append(
            _build_stream_groups(row_of(ek[m0]), dl[m0]))
        ek1 = ek[m0 == False]  # noqa: E712
        xe = np.concatenate([x[src[ek1]], ea[ek1]], axis=1)  # [n1, 7]
        dl1 = dl[m0 == False]  # noqa: E712
        o = np.argsort(dl1, kind="stable")
        per_core_f1.append({"dloc": dl1[o], "extra": xe[o]})

    def finish_phase(groups_by_core, unit):
        """groups_by_core: [NCORES][NRES] dicts. unit: pad granularity
        (GIDX_N for gather streams). Produces uniform instruction counts,
        per-core idx/dloc tables and the shared window schedule."""
        ph = {}
        n_instr = []
        for g in range(NRES):
            mx = max(len(groups_by_core[k][g]["idx"]) for k in range(NCORES))
            n_instr.append(max(1, math.ceil(mx / unit)))
        ph["n_instr"] = n_instr
        nch_g = [ni * (unit // P) for ni in n_instr]
        ph["nch_g"] = nch_g
        # per-core padded dloc [group][core][nch, P]
        dl_pad = []
        for g in range(NRES):
            percore = []
            for k in range(NCORES):
                dl = groups_by_core[k][g]["dloc"]
                buf = np.full(nch_g[g] * P, -1.0, np.float64)
                buf[:len(dl)] = dl
                percore.append(buf.reshape(nch_g[g], P))
            dl_pad.append(percore)
        # window schedule (uniform across cores)
        sched = []
        for g in range(NRES):
            mns = np.full((NCORES, nch_g[g]), np.inf)
            mxs = np.full((NCORES, nch_g[g]), -np.inf)
            for k in range(NCORES):
                mns[k], mxs[k] = _chunk_minmax(dl_pad[g][k])
            mn = mns.min(axis=0)
            mx = mxs.max(axis=0)
            wfirst = np.where(np.isfinite(mn), mn // WSL, -1).astype(np.int64)
            wlast = np.where(np.isfinite(mx), mx // WSL, -2).astype(np.int64)
            span = (wlast - wfirst + 1).clip(min=0)
            assert span.max(initial=0) <= JMAX, f"window span {span.max()}"
            wl = {}
            for i in range(nch_g[g]):
                for w in range(wfirst[i], wlast[i] + 1):
                    wl.setdefault(w, []).append(i)
            # per chunk: list of (w, j, start, stop)
            chunks = []
            for i in range(nch_g[g]):
                lst = []
                for w in range(wfirst[i], wlast[i] + 1):
                    lst.append((int(w), int(w - wfirst[i]),
                                wl[w][0] == i, wl[w][-1] == i))
                chunks.append(lst)
            sched.append({"chunks": chunks, "wfirst": wfirst})
        ph["sched"] = sched
        # per-core relative dloc columns [128, sum(nch_g)] f32
        dloc_cols = []
        for k in range(NCORES):
            cols = []
            for g in range(NRES):
                wf = sched[g]["wfirst"]
                rel = dl_pad[g][k] - (wf[:, None] * WSL)
                rel[dl_pad[g][k] < 0] = -1.0
                cols.append(rel.reshape(nch_g[g], P).T)
            dloc_cols.append(
                np.ascontiguousarray(np.concatenate(cols, axis=1)
                                     .astype(np.float32)))
        ph["dloc"] = dloc_cols
        # per-core wrapped idx tables [128, sum(n_instr)*ICOL] i16
        idx_cols = []
        for k in range(NCORES):
            cols = []
            for g in range(NRES):
                idx = groups_by_core[k][g]["idx"]
                buf = np.zeros(n_instr[g] * unit, np.int64)  # pad -> row 0
                buf[:len(idx)] = idx
                for ii in range(n_instr[g]):
                    cols.append(_wrap_idx16(buf[ii * unit:(ii + 1) * unit]))
            idx_cols.append(np.ascontiguousarray(np.concatenate(cols, axis=1)))
        ph["ga"] = idx_cols
        return ph

    plan.loop = finish_phase(per_core_loop, GIDX_N)
    plan.f0 = finish_phase(per_core_f0, GIDX_N)

    # fin stream 1 (no gather): chunk/window schedule + packed [7, C1*P] cols
    C1 = max(1, max(math.ceil(len(per_core_f1[k]["dloc"]) / P)
                    for k in range(NCORES)))
    C1 = math.ceil(C1 / GCH) * GCH   # round to full 2048-col batches
    plan.C1 = C1
    dl_pad1, mns, mxs = [], np.full((NCORES, C1), np.inf), \
        np.full((NCORES, C1), -np.inf)
    for k in range(NCORES):
        dl = per_core_f1[k]["dloc"]
        buf = np.full(C1 * P, -1.0, np.float64)
        buf[:len(dl)] = dl
        dl_pad1.append(buf.reshape(C1, P))
        mns[k], mxs[k] = _chunk_minmax(dl_pad1[k])
    mn, mx = mns.min(axis=0), mxs.max(axis=0)
    wfirst = np.where(np.isfinite(mn), mn // WSL, -1).astype(np.int64)
    wlast = np.where(np.isfinite(mx), mx // WSL, -2).astype(np.int64)
    assert (wlast - wfirst + 1).clip(min=0).max(initial=0) <= JMAX
    wl = {}
    for i in range(C1):
        for w in range(wfirst[i], wlast[i] + 1):
            wl.setdefault(w, []).append(i)
    chunks1 = []
    for i in range(C1):
        lst = []
        for w in range(wfirst[i], wlast[i] + 1):
            lst.append((int(w), int(w - wfirst[i]),
                        wl[w][0] == i, wl[w][-1] == i))
        chunks1.append(lst)
    plan.f1_sched = chunks1
    plan.f1_dloc, plan.f1_xe = [], []
    for k in range(NCORES):
        rel = dl_pad1[k] - (wfirst[:, None] * WSL)
        rel[dl_pad1[k] < 0] = -1.0
        plan.f1_dloc.append(np.ascontiguousarray(
            rel.reshape(C1, P).T.astype(np.float32)))
        xe = np.zeros((C1 * P, 7), np.float32)
        n1 = len(per_core_f1[k]["dloc"])
        xe[:n1] = per_core_f1[k]["extra"]
        plan.f1_xe.append(np.ascontiguousarray(xe.T.astype(BF16NP)))

    # init: [7, SHP] = [x[src[slot]], ea[slot]] per core
    plan.xe_init = []
    for k in range(NCORES):
        sl = np.arange(k * NSH, (k + 1) * NSH)
        xe = np.zeros((SHP, 7), np.float32)
        xe[:NSH, :4] = x[src[sl]]
        xe[:NSH, 4:] = ea[sl]
        plan.xe_init.append(np.ascontiguousarray(xe.T.astype(BF16NP)))

    # node features for the final W_a matmul + pooling info
    plan.xT = []
    plan.batchloc = []
    plan.g_bases = []
    for k in range(NCORES):
        xs = np.zeros((SHP, 4), np.float32)
        xs[:NSH] = x[k * NSH:(k + 1) * NSH]
        plan.xT.append(np.ascontiguousarray(xs.T.astype(BF16NP)))
        gb = int(batch[k * NSH])
        ge = int(batch[(k + 1) * NSH - 1])
        assert ge - gb < plan.GW, f"graph span {ge - gb} >= {plan.GW}"
        plan.g_bases.append(gb)
        bl = np.full((SHP,), -1.0, np.float32)
        bl[:NSH] = batch[k * NSH:(k + 1) * NSH] - gb
        plan.batchloc.append(
            np.ascontiguousarray(bl.reshape(NB, P).T))

    plan.nA_L = sum(plan.loop["n_instr"])
    plan.nA_F = sum(plan.f0["n_instr"])
    plan.ncols_L = sum(plan.loop["nch_g"])
    plan.ncols_F = sum(plan.f0["nch_g"])
    return plan


# ----------------------------------------------------------------------------
# device kernel
# ----------------------------------------------------------------------------

def _build(plan, split=True):
    NB, SHP, TBL, NW = plan.NB, plan.SHP, plan.TBL, plan.NW
    G, GW = plan.G, plan.GW
    depth = plan.depth
    TBL4 = TBL // NRES

    nc = bacc.Bacc(num_devices=NCORES)

    def din(name, shape, dt=F32):
        return nc.declare_dram_parameter(name, list(shape), dt, isOutput=False)

    WmT = din("WmT", [H, H], BF16)
    Wi7T = din("Wi7T", [7, H], BF16)
    WaxT = din("WaxT", [4, H], BF16)
    WahT = din("WahT", [H, H], BF16)
    W1T = din("W1T", [H, 4 * H], BF16)
    W2T = din("W2T", [4 * H, H], BF16)
    WlastT = din("WlastT", [H, 1])
    b1r = din("b1r", [H, 4])
    b2r = din("b2r", [H, 1])
    blast = din("blast", [1, 1])
    iotaWJ = din("iotaWJ", [P, JMAX * WSL])
    iotaG = din("iotaG", [P, GW])
    ident = din("ident", [P, P])
    xeI_in = din("xeI", [7, SHP], BF16)
    xeF_in = din("xeF", [7, plan.C1 * P], BF16)
    xT_in = din("xT", [4, SHP], BF16)
    batchloc_in = din("batchloc", [P, NB])
    gaL_in = din("gaL", [P, plan.nA_L * ICOL], I16)
    gaF_in = din("gaF", [P, plan.nA_F * ICOL], I16)
    dlocL_in = din("dlocL", [P, plan.ncols_L])
    dlocF_in = din("dlocF", [P, plan.ncols_F])
    dlocF1_in = din("dlocF1", [P, plan.C1])

    out_ext = nc.declare_dram_parameter("out", [G, 1], F32, isOutput=True)

    RG = list(range(NCORES))

    with tile.TileContext(nc) as tc:
        nc.gpsimd.load_library(library_config.mlp)
        with (
            tc.tile_pool(name="cp", bufs=1) as cp,
            tc.tile_pool(name="sb", bufs=3) as sb,
            tc.tile_pool(name="ps", bufs=2, space="PSUM") as ps,
            tc.tile_pool(name="dr", bufs=1, space="DRAM") as dr,
        ):
            def cload(name, src_t):
                tl = cp.tile([src_t.shape[0], src_t.shape[1]], src_t.dtype,
                             name=name)
                nc.sync.dma_start(out=tl[:], in_=src_t[:, :])
                return tl

            WmT_s = cload("WmT_s", WmT)
            Wi7T_s = cload("Wi7T_s", Wi7T)
            WaxT_s = cload("WaxT_s", WaxT)
            WahT_s = cload("WahT_s", WahT)
            W1T_s = cload("W1T_s", W1T)
            W2T_f = []
            for f in range(4):
                tl = cp.tile([P, H], BF16, name=f"W2T_{f}")
                nc.sync.dma_start(out=tl[:], in_=W2T[f * P:(f + 1) * P, :])
                W2T_f.append(tl)
            WlastT_s = cload("WlastT_s", WlastT)
            b1r_s = cload("b1r_s", b1r)
            b2r_s = cload("b2r_s", b2r)
            blast_s = cload("blast_s", blast)
            iotaWJ_s = cload("iotaWJ_s", iotaWJ)
            iotaG_s = cload("iotaG_s", iotaG)
            ident_s = cload("ident_s", ident)
            batchloc_s = cload("batchloc_s", batchloc_in)
            gaL_s = cload("gaL_s", gaL_in)
            gaF_s = cload("gaF_s", gaF_in)
            dlocL_s = cload("dlocL_s", dlocL_in)
            dlocF_s = cload("dlocF_s", dlocF_in)
            dlocF1_s = cload("dlocF1_s", dlocF1_in)

            hA_T = cp.tile([P, SHP], F32, name="hA_T")
            nacc_T = cp.tile([P, SHP], BF16, name="nacc_T")
            nc.vector.memset(nacc_T[:], 0.0)
            tab_tile = cp.tile([P, NB, H], BF16, name="tab_tile")
            gfull = cp.tile([P, G], F32, name="gfull")

            loc = dr.tile([SHP, H], BF16, name="loc")
            tbls = [dr.tile([TBL, H], BF16, name=f"tbl{i}",
                            addr_space="Shared") for i in range(depth + 1)]
            gwin_local = dr.tile([P, GW], F32, name="gwin_local")
            gwin_all = dr.tile([NCORES * P, GW], F32, name="gwin_all",
                               addr_space="Shared")

            def allgather(local, table):
                nc.gpsimd.collective_compute(
                    "AllGather", mybir.AluOpType.bypass,
                    replica_groups=[RG], ins=[local[:]], outs=[table[:]])

            def gather(dst_ap, tbl_t, res, idx_sb, inst_col):
                base = tbl_t[:]
                in_ap = bass.AP(base.tensor, base.offset + res * H,
                                [[NRES * H, TBL4], [1, H]])
                nc.gpsimd.dma_gather(
                    out_ap=dst_ap, in_ap=in_ap,
                    idxs_ap=idx_sb[:, inst_col * ICOL:(inst_col + 1) * ICOL],
                    num_idxs=GIDX_N, num_idxs_reg=GIDX_N,
                    elem_size=H, elem_step=NRES * H, single_packet=False)

            # ------------- scatter machinery (shared schedule walker) -------
            def scatter_chunks(chunk_iter, acc):
                """chunk_iter yields (msg_tile, cslice, dloc_col, wlist).
                One-hot matmul per (chunk, window) into a fresh PSUM tile,
                immediately added into `acc` [P, SHP]."""
                for msg, csl, dcol, wlist in chunk_iter:
                    for (w, j, st, sp) in wlist:
                        oh = sb.tile([P, WSL], BF16, name="oh", tag="oh",
                                     bufs=4)
                        nc.vector.tensor_tensor(
                            out=oh[:],
                            in0=dcol.to_broadcast([P, WSL]),
                            in1=iotaWJ_s[:, j * WSL:(j + 1) * WSL],
                            op=mybir.AluOpType.is_equal)
                        pw = ps.tile([P, WSL], F32, name="pw", tag="pacc",
                                     space="PSUM", bufs=3)
                        nc.tensor.matmul(out=pw[:], lhsT=msg[csl],
                                         rhs=oh[:], start=True, stop=True)
                        a = acc[:, w * WSL:(w + 1) * WSL]
                        nc.vector.tensor_add(out=a, in0=a, in1=pw[:])

            def gather_phase_chunks(ph, dloc_s, ga_s, tbl_t):
                """Generator of scatter_chunks items for a gather phase.
                Gathers are emitted with lookahead 2 so that tile-pool buffer
                reuse (bufs=3) never outruns already-emitted consumers."""
                LA = 2
                icol = 0
                col0 = 0
                for g in range(NRES):
                    sched = ph["sched"][g]["chunks"]
                    nin = ph["n_instr"][g]
                    tiles = [None] * nin

                    def emit(ii, icol0=icol):
                        gt = sb.tile([P, GCH, H], BF16, name="gt", tag="gat")
                        gather(gt[:], tbl_t, g, ga_s, icol0 + ii)
                        tiles[ii] = gt
                    for ii in range(min(LA, nin)):
                        emit(ii)
                    for ii in range(nin):
                        if ii + LA < nin:
                            emit(ii + LA)
                        for c in range(GCH):
                            i = ii * GCH + c
                            wlist = sched[i]
                            if not wlist:
                                continue
                            yield (tiles[ii], np.s_[:, c, :],
                                   dloc_s[:, col0 + i:col0 + i + 1], wlist)
                    icol += nin
                    col0 += ph["nch_g"][g]

            # ---------------- init ----------------
            for bi in range(math.ceil(NB / GCH)):
                b0 = bi * GCH
                b1_ = min(NB, b0 + GCH)
                xei = sb.tile([7, GIDX_N], BF16, name="xei", tag="xet",
                              bufs=2)
                nc.sync.dma_start(
                    out=xei[:, :(b1_ - b0) * P],
                    in_=xeI_in[:, b0 * P:b1_ * P])
                for b in range(b0, b1_):
                    pi = ps.tile([P, H], F32, name="pi", tag="ptmp",
                                 space="PSUM")
                    nc.tensor.matmul(out=pi[:], lhsT=Wi7T_s[:],
                                     rhs=xei[:, (b - b0) * P:(b - b0 + 1) * P],
                                     start=True, stop=True)
                    nc.scalar.activation(out=hA_T[:, b * P:(b + 1) * P],
                                         in_=pi[:], func=_relu())

            # ---------------- message-passing iterations ----------------
            for it in range(depth):
                # mA = relu(W_m h) -> bf16 table tile -> DRAM -> AllGather
                for b in range(NB):
                    hb = sb.tile([P, P], BF16, name="hb", tag="hb")
                    nc.vector.tensor_copy(out=hb[:],
                                          in_=hA_T[:, b * P:(b + 1) * P])
                    pm = ps.tile([P, H], F32, name="pm", tag="ptmp",
                                 space="PSUM")
                    nc.tensor.matmul(out=pm[:], lhsT=hb[:], rhs=WmT_s[:],
                                     start=True, stop=True)
                    nc.scalar.activation(out=tab_tile[:, b, :], in_=pm[:],
                                         func=_relu())
                nc.sync.dma_start(
                    out=loc[:, :].rearrange("(p b) h -> p (b h)", p=P),
                    in_=tab_tile[:])
                allgather(loc, tbls[it])
                scatter_chunks(
                    gather_phase_chunks(plan.loop, dlocL_s, gaL_s, tbls[it]),
                    hA_T)

            # ---------------- final aggregation ----------------
            # h_final table (transposed state) -> DRAM -> AllGather
            for b in range(NB):
                pt = ps.tile([P, H], F32, name="pt", tag="ptmp", space="PSUM")
                nc.tensor.transpose(out=pt[:], in_=hA_T[:, b * P:(b + 1) * P],
                                    identity=ident_s[:])
                nc.vector.tensor_copy(out=tab_tile[:, b, :], in_=pt[:])
            nc.sync.dma_start(
                out=loc[:, :].rearrange("(p b) h -> p (b h)", p=P),
                in_=tab_tile[:])
            allgather(loc, tbls[depth])

            # stream 1: h0 recompute for edge ids >= N (overlaps AG + gathers)
            def f1_chunks():
                nbatch = plan.C1 // GCH
                for bi in range(nbatch):
                    xet = sb.tile([7, GIDX_N], BF16, name="xet", tag="xet",
                                  bufs=2)
                    nc.sync.dma_start(
                        out=xet[:],
                        in_=xeF_in[:, bi * GIDX_N:(bi + 1) * GIDX_N])
                    for c in range(GCH):
                        i = bi * GCH + c
                        wlist = plan.f1_sched[i]
                        if not wlist:
                            continue
                        ph0 = ps.tile([P, H], F32, name="ph0", tag="ptmp",
                                      space="PSUM")
                        nc.tensor.matmul(out=ph0[:],
                                         lhsT=xet[:, c * P:(c + 1) * P],
                                         rhs=Wi7T_s[:], start=True, stop=True)
                        msg = sb.tile([P, H], BF16, name="msg", tag="msg",
                                      bufs=4)
                        nc.scalar.activation(out=msg[:], in_=ph0[:],
                                             func=_relu())
                        yield (msg, np.s_[:, :],
                               dlocF1_s[:, i:i + 1], wlist)

            # emit: f0 gathers first (Pool/DMA), then f1 (PE) runs under them,
            # then f0 scatter consumes the gathered tiles.
            f0_iter = gather_phase_chunks(plan.f0, dlocF_s, gaF_s, tbls[depth])
            scatter_chunks(f1_chunks(), nacc_T)
            scatter_chunks(f0_iter, nacc_T)

            # ---------------- node_emb + pooling ----------------
            gps = ps.tile([P, GW], F32, name="gps", tag="gps", space="PSUM",
                          bufs=1)
            xt_tiles = {}
            for b in range(NB):
                if b % GCH == 0:
                    b0 = b
                    b1_ = min(NB, b0 + GCH)
                    xtt = sb.tile([4, GIDX_N], BF16, name="xtt", tag="xet",
                                  bufs=2)
                    nc.sync.dma_start(
                        out=xtt[:, :(b1_ - b0) * P],
                        in_=xT_in[:, b0 * P:b1_ * P])
                p2 = ps.tile([P, H], F32, name="p2", tag="ptmp", space="PSUM")
                nc.tensor.matmul(out=p2[:], lhsT=nacc_T[:, b * P:(b + 1) * P],
                                 rhs=WahT_s[:], start=True, stop=False)
                nc.tensor.matmul(out=p2[:],
                                 lhsT=xtt[:, (b - b0) * P:(b - b0 + 1) * P],
                                 rhs=WaxT_s[:], start=False, stop=True)
                ne2 = sb.tile([P, H], BF16, name="ne2", tag="msg", bufs=4)
                nc.scalar.activation(out=ne2[:], in_=p2[:], func=_relu())
                ohg = sb.tile([P, GW], BF16, name="ohg", tag="ohg")
                nc.vector.tensor_tensor(
                    out=ohg[:],
                    in0=batchloc_s[:, b:b + 1].to_broadcast([P, GW]),
                    in1=iotaG_s[:], op=mybir.AluOpType.is_equal)
                nc.tensor.matmul(out=gps[:], lhsT=ne2[:], rhs=ohg[:],
                                 start=(b == 0), stop=(b == NB - 1))

            tgw = sb.tile([P, GW], F32, name="tgw", tag="ohg")
            nc.vector.tensor_copy(out=tgw[:], in_=gps[:])
            nc.sync.dma_start(out=gwin_local[:, :], in_=tgw[:])
            allgather(gwin_local, gwin_all)
            nc.vector.memset(gfull[:], 0.0)
            for j in range(NCORES):
                wj = min(GW, G - plan.g_bases[j])
                tw = sb.tile([P, GW], F32, name="twj", tag="ohg")
                nc.sync.dma_start(out=tw[:], in_=gwin_all[j * P:(j + 1) * P, :])
                nc.vector.tensor_add(
                    out=gfull[:, plan.g_bases[j]:plan.g_bases[j] + wj],
                    in0=gfull[:, plan.g_bases[j]:plan.g_bases[j] + wj],
                    in1=tw[:, :wj])

            # ---------------- FFN (replicated on all cores) ----------------
            NGC = math.ceil(G / 512)
            for gc in range(NGC):
                g0, g1 = gc * 512, min((gc + 1) * 512, G)
                pz2 = ps.tile([P, 512], F32, name="pz2", tag="pz2",
                              space="PSUM", bufs=1)
                gf16 = sb.tile([P, 512], BF16, name="gf16", tag="z1", bufs=2)
                nc.vector.tensor_copy(out=gf16[:, :g1 - g0],
                                      in_=gfull[:, g0:g1])
                for f in range(4):
                    pz = ps.tile([P, 512], F32, name="pz", tag="ptmp",
                                 space="PSUM")
                    nc.tensor.matmul(out=pz[:, :g1 - g0],
                                     lhsT=W1T_s[:, f * P:(f + 1) * P],
                                     rhs=gf16[:, :g1 - g0], start=True, stop=True)
                    z1 = sb.tile([P, 512], BF16, name="z1", tag="z1", bufs=2)
                    nc.scalar.activation(out=z1[:, :g1 - g0],
                                         in_=pz[:, :g1 - g0], func=_relu(),
                                         bias=b1r_s[:, f:f + 1])
                    nc.tensor.matmul(out=pz2[:, :g1 - g0], lhsT=W2T_f[f][:],
                                     rhs=z1[:, :g1 - g0], start=(f == 0),
                                     stop=(f == 3))
                z2 = sb.tile([P, 512], F32, name="z2", tag="z1", bufs=2)
                nc.vector.tensor_add(
                    out=z2[:, :g1 - g0], in0=pz2[:, :g1 - g0],
                    in1=b2r_s[:, 0:1].to_broadcast([P, g1 - g0]))
                po = ps.tile([1, 512], F32, name="po", tag="ptmp",
                             space="PSUM")
                nc.tensor.matmul(out=po[:, :g1 - g0], lhsT=WlastT_s[:],
                                 rhs=z2[:, :g1 - g0], start=True, stop=True)
                oc = sb.tile([1, 512], F32, name="oc", tag="oc", bufs=2)
                nc.vector.tensor_add(
                    out=oc[:, :g1 - g0], in0=po[:, :g1 - g0],
                    in1=blast_s[0:1, 0:1].to_broadcast([1, g1 - g0]))
                nc.sync.dma_start(out=out_ext[g0:g1, :], in_=oc[:, :g1 - g0])

    nc.compile()
    if split:
        _split_excess_waits(nc)
    return nc


def _split_excess_waits(nc, max_waits=1):
    k = 0
    for f in nc.m.functions:
        for bb in f.blocks:
            new = []
            for ins in bb.instructions:
                si = ins.sync_info
                if si is not None and len(si.on_wait) > max_waits:
                    waits = list(si.on_wait)
                    for w in waits[:-max_waits]:
                        nop = mybir.InstNoOp(name=f"I-waitsplit-{k}",
                                             engine=ins.engine)
                        k += 1
                        nop.sync_info = mybir.SyncInfo(on_wait=[w],
                                                       on_update=[])
                        new.append(nop)
                    si.on_wait = waits[-max_waits:]
                new.append(ins)
            bb.instructions = new
    return k


# ----------------------------------------------------------------------------
# inputs
# ----------------------------------------------------------------------------

def _in_maps(plan, weights):
    com = {
        "WmT": np.ascontiguousarray(weights["W_m"].T).astype(BF16NP),
        "Wi7T": np.ascontiguousarray(weights["W_i"].T).astype(BF16NP),
        "WaxT": np.ascontiguousarray(weights["W_a"][:, :4].T).astype(BF16NP),
        "WahT": np.ascontiguousarray(weights["W_a"][:, 4:].T).astype(BF16NP),
        "W1T": np.ascontiguousarray(weights["W1"].T).astype(BF16NP),
        "W2T": np.ascontiguousarray(weights["W2"].T).astype(BF16NP),
        "WlastT": np.ascontiguousarray(weights["W_last"].T),
        "b1r": np.ascontiguousarray(weights["b1"].reshape(4, H).T),
        "b2r": weights["b2"].reshape(H, 1).copy(),
        "blast": weights["b_last"].reshape(1, 1).copy(),
        "iotaWJ": np.tile(np.arange(JMAX * WSL, dtype=np.float32), (P, 1)),
        "iotaG": np.tile(np.arange(plan.GW, dtype=np.float32), (P, 1)),
        "ident": np.eye(P, dtype=np.float32),
    }
    maps = []
    for k in range(NCORES):
        m = dict(com)
        m["xeI"] = plan.xe_init[k]
        m["xeF"] = plan.f1_xe[k]
        m["xT"] = plan.xT[k]
        m["batchloc"] = plan.batchloc[k]
        m["gaL"] = plan.loop["ga"][k]
        m["gaF"] = plan.f0["ga"][k]
        m["dlocL"] = plan.loop["dloc"][k]
        m["dlocF"] = plan.f0["dloc"][k]
        m["dlocF1"] = plan.f1_dloc[k]
        maps.append(m)
    return maps


def _prep_all(x, edge_index, edge_attr, batch, depth, weights, G):
    plan = _host_prep(np.asarray(x, np.float32), np.asarray(edge_index),
                      np.asarray(edge_attr, np.float32), np.asarray(batch),
                      int(depth), G)
    maps = _in_maps(plan, weights)
    return plan, maps


def kernel(x, edge_index, edge_attr, batch, depth,
           W_i, W_m, W_a, W1, b1, W2, b2, W_last, b_last):
    weights = {
        "W_i": np.asarray(W_i, np.float32), "W_m": np.asarray(W_m, np.float32),
        "W_a": np.asarray(W_a, np.float32), "W1": np.asarray(W1, np.float32),
        "b1": np.asarray(b1, np.float32), "W2": np.asarray(W2, np.float32),
        "b2": np.asarray(b2, np.float32),
        "W_last": np.asarray(W_last, np.float32),
        "b_last": np.asarray(b_last, np.float32),
    }
    G = 2048
    plan, maps = _prep_all(x, edge_index, edge_attr, batch, depth, weights, G)
    nc = _build(plan, split=True)
    res = run_bass_kernel_spmd(nc, maps, list(range(NCORES)))
    return np.asarray(res.results[0]["out"]).reshape(G, 1).astype(np.float32)
